# revision 1
# baseline (speedup 1.0000x reference)
"""GNN message passing (copy_src + segment_sum + Linear + ReLU) on 8 TRN2 cores.

Structure: dst nodes are bin-packed (host side) into 392 windows = 8 cores
x 49 slots, <=128 nodes per window, with per-slot uniform edge-tile
capacities (KA_s, KB_s) shared by all cores so the instruction stream is
SPMD-identical. Each core gathers the src rows of its edges from a bf16
replica of the feature table (split in two halves at node 25000 for the
int16 gather-index range) with 1024-index single-packet dma_gather calls
rotated over 4 SWDGE queues — descriptor generation is the bottleneck and
parallelizes ~4x across queues (measured 2.2-2.6 ns/idx vs 4.6 at 2 queues;
bigger calls, multi-packet mode, or a bigger descriptor carveout are all
2.5x slower or wedge the device). The per-edge one-hot(dst lane) matrices
are precomputed on the host and streamed from HBM via HWDGE — building them
with a DVE tensor_tensor stalls SWDGE descriptor generation (shared SBUF
port with the Q7 descriptor rings). PE matmuls accumulate
aggT[f, lane] += msgs[e, f].T @ onehot[e, lane] in PSUM (fp32). Node update
per window: ACT copies aggT to bf16, matmul with W^T + K=1 bias matmul,
ACT ReLU, DMA out fp32 on the ACT HWDGE ring. Host assembles the full
output from the window node lists. No collectives.

Self-contained: shapes hardcoded for feature[50000,128], src/dst[640000],
W[128,128], b[128].
"""
import numpy as np
import ml_dtypes

import concourse.bacc as bacc
import concourse.tile as tile
from concourse import mybir
from concourse.bass_utils import run_bass_kernel_spmd

P = 128
N_NODES = 50000
N_EDGES = 640000
VHALF = 25000
NC = 8
W_SLOTS = 49
NBINS = NC * W_SLOTS                 # 392 windows
CALL_TILES = 8                       # 1024 idx per dma_gather (ring capacity)
NQ = 4                               # SWDGE queues
BATCH_SLOTS = 8

F32 = mybir.dt.float32
BF16 = mybir.dt.bfloat16
I16 = mybir.dt.int16
BF = ml_dtypes.bfloat16


def _slot_caps(extra14):
    """Per-slot (KA, KB): slots 0..24 are (7,6), 25..48 are (6,7); the first
    `extra14` slots of each group get +1 on the minor side (K=14)."""
    caps = []
    for s in range(W_SLOTS):
        if s < 25:
            ka, kb = 7, 6
            if s < extra14:
                kb += 1
        else:
            ka, kb = 6, 7
            if s - 25 < extra14:
                ka += 1
        caps.append((ka, kb))
    return caps


def _pack_nodes(da, db, caps):
    """Assign all nodes to NBINS bins with per-bin caps (128 nodes,
    KA*128 A-edges, KB*128 B-edges). Vectorized greedy, high-degree first.
    Returns bins: list of node-id arrays, or None if packing fails."""
    n = da + db
    order = np.argsort(-(n.astype(np.int64) * 4 + (da > db)))
    a_cap = np.array([caps[b % W_SLOTS][0] * P for b in range(NBINS)],
                     dtype=np.int64)
    b_cap = np.array([caps[b % W_SLOTS][1] * P for b in range(NBINS)],
                     dtype=np.int64)
    a_left = a_cap.copy()
    b_left = b_cap.copy()
    n_left = np.full(NBINS, P, dtype=np.int64)
    assign = np.empty(N_NODES, dtype=np.int64)
    for node in order:
        ai, bi = da[node], db[node]
        feas = (n_left > 0) & (a_left >= ai) & (b_left >= bi)
        if not feas.any():
            return None
        # prefer the bin whose remaining a:b slack best matches this node's
        # skew, with a tie-break toward emptier bins
        slack = np.minimum(a_left - ai, b_left - bi).astype(np.float64)
        score = slack + 0.25 * (a_left + b_left - ai - bi)
        score[~feas] = -1e18
        bsel = int(np.argmax(score))
        assign[node] = bsel
        a_left[bsel] -= ai
        b_left[bsel] -= bi
        n_left[bsel] -= 1
    bins = [np.where(assign == b)[0] for b in range(NBINS)]
    return bins


def _make_plan(src, dst):
    src = np.asarray(src, dtype=np.int64)
    dst = np.asarray(dst, dtype=np.int64)
    isa = src < VHALF
    da = np.bincount(dst[isa], minlength=N_NODES)
    db = np.bincount(dst[~isa], minlength=N_NODES)

    for extra14 in (0, 2, 4, 8, 16, 25):
        caps = _slot_caps(extra14)
        bins = _pack_nodes(da, db, caps)
        if bins is not None:
            break
    else:
        raise RuntimeError("node packing failed")

    KA = np.array([c[0] for c in caps], dtype=np.int64)
    KB = np.array([c[1] for c in caps], dtype=np.int64)
    TA = int(KA.sum())
    TB = int(KB.sum())
    T_tot = TA + TB
    a_base = np.concatenate([[0], np.cumsum(KA)])
    b_base = np.concatenate([[0], np.cumsum(KB)])

    # per-edge bin and lane-in-bin
    node_bin = np.empty(N_NODES, dtype=np.int64)
    node_lane = np.empty(N_NODES, dtype=np.int64)
    for bid, nodes in enumerate(bins):
        node_bin[nodes] = bid
        node_lane[nodes] = np.arange(len(nodes))

    ebin = node_bin[dst]
    # order edges by (bin, half, src) for locality and contiguous fill
    order = np.lexsort((src, ~isa, ebin))
    e_bin = ebin[order]
    e_half = (~isa)[order]            # 0 = A, 1 = B
    e_src = src[order]
    e_lane = node_lane[dst[order]]

    # batches of BATCH_SLOTS slots; per batch, A-calls and B-calls chunk the
    # batch's contiguous A/B tile regions into <=CALL_TILES-tile calls.
    # dstloc columns are stored in BATCH order (batch's A tiles then B tiles)
    # so the one-hot is built with ONE tensor_tensor per batch.
    batches = []
    s = 0
    t_base = 0
    dstcol_a = np.zeros(W_SLOTS, dtype=np.int64)   # dstloc col of slot's tile 0 (A)
    dstcol_b = np.zeros(W_SLOTS, dtype=np.int64)
    while s < W_SLOTS:
        s1 = min(s + BATCH_SLOTS, W_SLOTS)
        slots = list(range(s, s1))
        ka_b = int(KA[s:s1].sum())
        kb_b = int(KB[s:s1].sum())
        wins = []
        pa, pb = 0, 0
        for w in slots:
            dstcol_a[w] = t_base + pa
            dstcol_b[w] = t_base + ka_b + pb
            cols = list(range(pa, pa + int(KA[w]))) + \
                   list(range(ka_b + pb, ka_b + pb + int(KB[w])))
            wins.append((w, cols))
            pa += int(KA[w])
            pb += int(KB[w])
        batches.append(dict(ka=ka_b, kb=kb_b, t_base=t_base,
                            a0=int(a_base[s]), b0=int(b_base[s]), wins=wins))
        t_base += ka_b + kb_b
        s = s1

    dstloc = np.full((NC, P, T_tot), -1.0, dtype=np.float64)
    idxA = np.zeros((NC, TA * P), dtype=np.int16)
    idxB = np.zeros((NC, TB * P), dtype=np.int16)

    # bincount per (bin, half)
    key = e_bin * 2 + e_half
    cnt = np.bincount(key, minlength=NBINS * 2)
    starts = np.concatenate([[0], np.cumsum(cnt)])
    for bid in range(NBINS):
        c, s = divmod(bid, W_SLOTS)
        for half in (0, 1):
            k = bid * 2 + half
            e0, e1 = starts[k], starts[k + 1]
            m = e1 - e0
            if m == 0:
                continue
            j = np.arange(m)
            if half == 0:
                assert m <= KA[s] * P, (bid, m, KA[s] * P)
                dstloc[c, j % P, dstcol_a[s] + j // P] = e_lane[e0:e1]
                idxA[c, a_base[s] * P + j] = e_src[e0:e1].astype(np.int16)
            else:
                assert m <= KB[s] * P, (bid, m, KB[s] * P)
                dstloc[c, j % P, dstcol_b[s] + j // P] = e_lane[e0:e1]
                idxB[c, b_base[s] * P + j] = (e_src[e0:e1] - VHALF).astype(np.int16)

    return dict(bins=bins, caps=caps, KA=KA, KB=KB, TA=TA, TB=TB,
                T_tot=T_tot, batches=batches, dstloc=dstloc,
                idxA=idxA, idxB=idxB)


def _wrap16(idx_flat):
    n = idx_flat.shape[0]
    arr = np.empty((16, n // 16), dtype=np.int16)
    j = np.arange(n)
    arr[j % 16, j // 16] = idx_flat
    return np.tile(arr, (8, 1))


def _build_nc(plan):
    TA, TB, T_tot = plan["TA"], plan["TB"], plan["T_tot"]
    # batch 0's index columns are stored first, in their own small region,
    # so the first gather calls only wait on a ~0.6us DMA instead of the
    # whole 10KB/partition index table
    ka0 = plan["batches"][0]["ka"]
    kb0 = plan["batches"][0]["kb"]
    plan["ka0"], plan["kb0"] = ka0, kb0
    # const fp32 column layout: [idxA_b0 | idxB_b0][idxA_rest | idxB_rest |
    # W^T | bias]
    c_ia0 = 0                               # idxA batch0 int16 [P, ka0*8]
    c_ib0 = c_ia0 + ka0 * 4                 # idxB batch0 int16 [P, kb0*8]
    c_first = c_ib0 + kb0 * 4               # end of first region
    c_ia = c_first                          # idxA rest int16
    c_ib = c_ia + (TA - ka0) * 4            # idxB rest int16
    c_wt = c_ib + (TB - kb0) * 4            # W^T bf16 [P, P]
    c_brow = c_wt + P // 2                  # bias row bf16 [1, P]
    c_tot = c_brow + P // 2
    plan["c_layout"] = (c_ia0, c_ib0, c_first, c_ia, c_ib, c_wt, c_brow, c_tot)

    nc = bacc.Bacc("TRN2", num_swdge_queues=NQ)
    featA = nc.declare_dram_parameter("featA", [VHALF, P], BF16, isOutput=False)
    featB = nc.declare_dram_parameter("featB", [N_NODES - VHALF, P], BF16,
                                      isOutput=False)
    oneh_d = nc.declare_dram_parameter("oneh", [P, T_tot, P], BF16,
                                       isOutput=False)
    consts = nc.declare_dram_parameter("consts", [P, c_tot], F32, isOutput=False)
    out = nc.declare_dram_parameter("out", [W_SLOTS * P, P], F32, isOutput=True)

    with tile.TileContext(nc) as tc:
        with (
            tc.tile_pool(name="const", bufs=1) as const_pool,
            tc.tile_pool(name="msgs", bufs=3) as msgs_pool,
            tc.tile_pool(name="oneh", bufs=3) as oneh_pool,
            tc.tile_pool(name="outp", bufs=3) as out_pool,
            tc.tile_pool(name="psum", bufs=4, space="PSUM") as psum_pool,
        ):
            # three-way consts load: batch 0's indices land first so the
            # gather stream starts immediately
            cs0 = const_pool.tile([P, c_first], F32, tag="cs_b0")
            nc.sync.dma_start(out=cs0[:], in_=consts[:, 0:c_first])
            cs = const_pool.tile([P, c_wt - c_first], F32, tag="cs_idx")
            nc.sync.dma_start(out=cs[:], in_=consts[:, c_first:c_wt])
            csm = const_pool.tile([P, c_tot - c_wt], F32, tag="cs_misc")
            nc.sync.dma_start(out=csm[:], in_=consts[:, c_wt:c_tot])
            idxA0_sb = cs0[:, c_ia0:c_ib0].bitcast(I16)
            idxB0_sb = cs0[:, c_ib0:c_first].bitcast(I16)
            idxA_sb = cs[:, 0:c_ib - c_ia].bitcast(I16)
            idxB_sb = cs[:, c_ib - c_ia:].bitcast(I16)
            wt_sb = csm[:, 0:c_brow - c_wt].bitcast(BF16)
            brow_sb = csm[0:1, c_brow - c_wt:].bitcast(BF16)
            ones_sb = const_pool.tile([1, P], BF16)
            nc.vector.memset(ones_sb[:], 1.0)

            gq = [0]

            def gather_calls(msgs, coff, k_tiles, feat, idx_sb, flat0):
                # chunk [0, k_tiles) into CALL_TILES-tile calls (full tiles
                # first; ring/packet capacity caps a call at 1024 indices)
                sizes = [min(CALL_TILES, k_tiles - o)
                         for o in range(0, k_tiles, CALL_TILES)]
                off = 0
                for nk in sizes:
                    nc.gpsimd.dma_gather(
                        out_ap=msgs[:, coff + off:coff + off + nk, :],
                        in_ap=feat[:],
                        idxs_ap=idx_sb[:, (flat0 + off) * 8:(flat0 + off + nk) * 8],
                        num_idxs=nk * P,
                        num_idxs_reg=nk * P,
                        elem_size=P,
                        queue_num=gq[0] % NQ,
                    )
                    gq[0] += 1
                    off += nk

            for bt in plan["batches"]:
                ka_b, kb_b = bt["ka"], bt["kb"]
                k_b = ka_b + kb_b
                msgs = msgs_pool.tile([P, k_b, P], BF16, tag="msgs")
                if bt["a0"] == 0:     # batch 0: its own early-loaded region
                    gather_calls(msgs, 0, ka_b, featA, idxA0_sb, 0)
                    gather_calls(msgs, ka_b, kb_b, featB, idxB0_sb, 0)
                else:
                    gather_calls(msgs, 0, ka_b, featA, idxA_sb,
                                 bt["a0"] - plan["ka0"])
                    gather_calls(msgs, ka_b, kb_b, featB, idxB_sb,
                                 bt["b0"] - plan["kb0"])

                onehot = oneh_pool.tile([P, k_b, P], BF16, tag="onehot")
                # host-precomputed one-hot, streamed via HWDGE (keeps DVE off
                # the SBUF port GpSimd's descriptor rings live on). Chunked
                # <=16 tiles per DMA so the prefetch trickles instead of a
                # 2.7MB burst that stalls the gather-ring drains on the
                # shared SDMA engines.
                for c0 in range(0, k_b, 16):
                    c1 = min(c0 + 16, k_b)
                    nc.sync.dma_start(
                        out=onehot[:, c0:c1, :],
                        in_=oneh_d[:, bt["t_base"] + c0:bt["t_base"] + c1, :])
                # one output DMA per batch (windows are contiguous out rows):
                # 7 HWDGE dispatches instead of 49 shortens the kernel tail
                nwin = len(bt["wins"])
                w0 = bt["wins"][0][0]
                out_b = out_pool.tile([P, nwin, P], F32, tag="out_b")
                for wi, (w, cols) in enumerate(bt["wins"]):
                    aggT_ps = psum_pool.tile([P, P], F32, tag="aggT")
                    for i, ccol in enumerate(cols):
                        nc.tensor.matmul(
                            out=aggT_ps[:],
                            lhsT=msgs[:, ccol, :],
                            rhs=onehot[:, ccol, :],
                            start=(i == 0),
                            stop=(i == len(cols) - 1),
                        )
                    aggT_sb = out_pool.tile([P, P], BF16, tag="aggT_sb")
                    nc.scalar.activation(out=aggT_sb[:], in_=aggT_ps[:],
                                         func=mybir.ActivationFunctionType.Copy)
                    out2_ps = psum_pool.tile([P, P], F32, tag="out2")
                    nc.tensor.matmul(out=out2_ps[:], lhsT=aggT_sb[:], rhs=wt_sb,
                                     start=True, stop=False)
                    nc.tensor.matmul(out=out2_ps[:], lhsT=ones_sb[:], rhs=brow_sb,
                                     start=False, stop=True)
                    nc.scalar.activation(out=out_b[:, wi, :], in_=out2_ps[:],
                                         func=mybir.ActivationFunctionType.Relu)
                nc.scalar.dma_start(
                    out=out[w0 * P:(w0 + nwin) * P, :]
                        .rearrange("(w p) f -> p w f", p=P),
                    in_=out_b[:])
    nc.finalize()
    return nc


_CACHE = {}


def _prepare(feature, src, dst, W, b):
    feature = np.asarray(feature, dtype=np.float32)
    W = np.asarray(W, dtype=np.float32)
    b = np.asarray(b, dtype=np.float32)
    key = (hash(np.asarray(src).tobytes()), hash(np.asarray(dst).tobytes()))
    if key not in _CACHE:
        plan = _make_plan(src, dst)
        nc = _build_nc(plan)
        _CACHE.clear()
        _CACHE[key] = (plan, nc)
    plan, nc = _CACHE[key]
    c_ia0, c_ib0, c_first, c_ia, c_ib, c_wt, c_brow, c_tot = plan["c_layout"]
    ka0, kb0 = plan["ka0"], plan["kb0"]
    TA, TB, T_tot = plan["TA"], plan["TB"], plan["T_tot"]
    featA = np.ascontiguousarray(feature[:VHALF].astype(BF))
    featB = np.ascontiguousarray(feature[VHALF:].astype(BF))
    lanes = np.arange(P, dtype=np.float64)
    in_maps = []
    for c in range(NC):
        consts = np.zeros((P, c_tot), dtype=np.float32)

        def put_bf16(col0, arr2d):
            # arr2d [rows, cols] bf16 -> fp32 columns starting at col0
            a = np.asarray(arr2d, dtype=BF)
            rows, cols = a.shape
            pad = (-cols) % 2
            if pad:
                a = np.concatenate([a, np.zeros((rows, pad), BF)], axis=1)
            a = np.ascontiguousarray(a)
            consts[:rows, col0:col0 + a.shape[1] // 2] = a.view(np.float32)

        wA = _wrap16(plan["idxA"][c]).view(np.float32)
        wB = _wrap16(plan["idxB"][c]).view(np.float32)
        consts[:, c_ia0:c_ib0] = wA[:, :ka0 * 4]
        consts[:, c_ib0:c_first] = wB[:, :kb0 * 4]
        consts[:, c_ia:c_ib] = wA[:, ka0 * 4:]
        consts[:, c_ib:c_wt] = wB[:, kb0 * 4:]
        put_bf16(c_wt, W.T.astype(BF))
        put_bf16(c_brow, b[None, :].astype(BF))
        oneh = (plan["dstloc"][c][:, :, None] == lanes[None, None, :]).astype(BF)
        in_maps.append({"featA": featA, "featB": featB, "consts": consts,
                        "oneh": np.ascontiguousarray(oneh)})
    return plan, nc, in_maps


def _assemble(plan, results):
    out_full = np.zeros((N_NODES, P), dtype=np.float32)
    for c in range(NC):
        oc = results[c]["out"]
        for s in range(W_SLOTS):
            nodes = plan["bins"][c * W_SLOTS + s]
            if len(nodes):
                out_full[nodes] = oc[s * P:s * P + len(nodes)]
    return out_full


def kernel(feature, src, dst, W, b):
    plan, nc, in_maps = _prepare(feature, src, dst, W, b)
    res = run_bass_kernel_spmd(nc, in_maps, list(range(NC)))
    return _assemble(plan, res.results)



# revision 4
# speedup vs baseline: 20.0562x; 20.0562x over previous
"""GNN message passing (copy_src + segment_sum + Linear + ReLU) on 8 TRN2 cores.

Structure: dst nodes are bin-packed (host side) into 392 windows = 8 cores
x 49 slots, <=128 nodes per window, with per-slot uniform edge-tile
capacities (KA_s, KB_s) shared by all cores so the instruction stream is
SPMD-identical. Each core gathers the src rows of its edges from a bf16
replica of the feature table (split in two halves at node 25000 for the
int16 gather-index range) with 1024-index single-packet dma_gather calls
rotated over 4 SWDGE queues — descriptor generation is the bottleneck and
parallelizes ~4x across queues (measured 2.2-2.6 ns/idx vs 4.6 at 2 queues;
bigger calls, multi-packet mode, or a bigger descriptor carveout are all
2.5x slower or wedge the device). The per-edge one-hot(dst lane) matrices
are precomputed on the host and streamed from HBM via HWDGE — building them
with a DVE tensor_tensor stalls SWDGE descriptor generation (shared SBUF
port with the Q7 descriptor rings). PE matmuls accumulate
aggT[f, lane] += msgs[e, f].T @ onehot[e, lane] in PSUM (fp32). Node update
per window: ACT copies aggT to bf16, matmul with W^T + K=1 bias matmul,
ACT ReLU, DMA out fp32 on the ACT HWDGE ring. Host assembles the full
output from the window node lists. No collectives.

Self-contained: shapes hardcoded for feature[50000,128], src/dst[640000],
W[128,128], b[128].
"""
import numpy as np
import ml_dtypes

import concourse.bacc as bacc
import concourse.tile as tile
from concourse import mybir
from concourse.bass_utils import run_bass_kernel_spmd

P = 128
N_NODES = 50000
N_EDGES = 640000
VHALF = 25000
NC = 8
W_SLOTS = 49
NBINS = NC * W_SLOTS                 # 392 windows
CALL_TILES = 8                       # 1024 idx per dma_gather (ring capacity)
NQ = 4                               # SWDGE queues
BATCH_SLOTS = 8

F32 = mybir.dt.float32
BF16 = mybir.dt.bfloat16
I16 = mybir.dt.int16
BF = ml_dtypes.bfloat16


def _slot_caps(extra14):
    """Per-slot (KA, KB): slots 0..24 are (7,6), 25..48 are (6,7); the first
    `extra14` slots of each group get +1 on the minor side (K=14)."""
    caps = []
    for s in range(W_SLOTS):
        if s < 25:
            ka, kb = 7, 6
            if s < extra14:
                kb += 1
        else:
            ka, kb = 6, 7
            if s - 25 < extra14:
                ka += 1
        caps.append((ka, kb))
    return caps


def _pack_nodes(da, db, caps):
    """Assign all nodes to NBINS bins with per-bin caps (128 nodes,
    KA*128 A-edges, KB*128 B-edges). Vectorized greedy, high-degree first.
    Returns bins: list of node-id arrays, or None if packing fails."""
    n = da + db
    order = np.argsort(-(n.astype(np.int64) * 4 + (da > db)))
    a_cap = np.array([caps[b % W_SLOTS][0] * P for b in range(NBINS)],
                     dtype=np.int64)
    b_cap = np.array([caps[b % W_SLOTS][1] * P for b in range(NBINS)],
                     dtype=np.int64)
    a_left = a_cap.copy()
    b_left = b_cap.copy()
    n_left = np.full(NBINS, P, dtype=np.int64)
    assign = np.empty(N_NODES, dtype=np.int64)
    for node in order:
        ai, bi = da[node], db[node]
        feas = (n_left > 0) & (a_left >= ai) & (b_left >= bi)
        if not feas.any():
            return None
        # prefer the bin whose remaining a:b slack best matches this node's
        # skew, with a tie-break toward emptier bins
        slack = np.minimum(a_left - ai, b_left - bi).astype(np.float64)
        score = slack + 0.25 * (a_left + b_left - ai - bi)
        score[~feas] = -1e18
        bsel = int(np.argmax(score))
        assign[node] = bsel
        a_left[bsel] -= ai
        b_left[bsel] -= bi
        n_left[bsel] -= 1
    bins = [np.where(assign == b)[0] for b in range(NBINS)]
    return bins


def _make_plan(src, dst):
    src = np.asarray(src, dtype=np.int64)
    dst = np.asarray(dst, dtype=np.int64)
    isa = src < VHALF
    da = np.bincount(dst[isa], minlength=N_NODES)
    db = np.bincount(dst[~isa], minlength=N_NODES)

    for extra14 in (0, 2, 4, 8, 16, 25):
        caps = _slot_caps(extra14)
        bins = _pack_nodes(da, db, caps)
        if bins is not None:
            break
    else:
        raise RuntimeError("node packing failed")

    KA = np.array([c[0] for c in caps], dtype=np.int64)
    KB = np.array([c[1] for c in caps], dtype=np.int64)
    TA = int(KA.sum())
    TB = int(KB.sum())
    T_tot = TA + TB
    a_base = np.concatenate([[0], np.cumsum(KA)])
    b_base = np.concatenate([[0], np.cumsum(KB)])

    # per-edge bin and lane-in-bin
    node_bin = np.empty(N_NODES, dtype=np.int64)
    node_lane = np.empty(N_NODES, dtype=np.int64)
    for bid, nodes in enumerate(bins):
        node_bin[nodes] = bid
        node_lane[nodes] = np.arange(len(nodes))

    ebin = node_bin[dst]
    # order edges by (bin, half, src) for locality and contiguous fill
    order = np.lexsort((src, ~isa, ebin))
    e_bin = ebin[order]
    e_half = (~isa)[order]            # 0 = A, 1 = B
    e_src = src[order]
    e_lane = node_lane[dst[order]]

    # batches of BATCH_SLOTS slots; per batch, A-calls and B-calls chunk the
    # batch's contiguous A/B tile regions into <=CALL_TILES-tile calls.
    # dstloc columns are stored in BATCH order (batch's A tiles then B tiles)
    # so the one-hot is built with ONE tensor_tensor per batch.
    batches = []
    s = 0
    t_base = 0
    dstcol_a = np.zeros(W_SLOTS, dtype=np.int64)   # dstloc col of slot's tile 0 (A)
    dstcol_b = np.zeros(W_SLOTS, dtype=np.int64)
    while s < W_SLOTS:
        s1 = min(s + BATCH_SLOTS, W_SLOTS)
        slots = list(range(s, s1))
        ka_b = int(KA[s:s1].sum())
        kb_b = int(KB[s:s1].sum())
        wins = []
        pa, pb = 0, 0
        for w in slots:
            dstcol_a[w] = t_base + pa
            dstcol_b[w] = t_base + ka_b + pb
            cols = list(range(pa, pa + int(KA[w]))) + \
                   list(range(ka_b + pb, ka_b + pb + int(KB[w])))
            wins.append((w, cols))
            pa += int(KA[w])
            pb += int(KB[w])
        batches.append(dict(ka=ka_b, kb=kb_b, t_base=t_base,
                            a0=int(a_base[s]), b0=int(b_base[s]), wins=wins))
        t_base += ka_b + kb_b
        s = s1

    dstloc = np.full((NC, P, T_tot), -1.0, dtype=np.float64)
    idxA = np.zeros((NC, TA * P), dtype=np.int16)
    idxB = np.zeros((NC, TB * P), dtype=np.int16)

    # bincount per (bin, half)
    key = e_bin * 2 + e_half
    cnt = np.bincount(key, minlength=NBINS * 2)
    starts = np.concatenate([[0], np.cumsum(cnt)])
    for bid in range(NBINS):
        c, s = divmod(bid, W_SLOTS)
        for half in (0, 1):
            k = bid * 2 + half
            e0, e1 = starts[k], starts[k + 1]
            m = e1 - e0
            if m == 0:
                continue
            j = np.arange(m)
            if half == 0:
                assert m <= KA[s] * P, (bid, m, KA[s] * P)
                dstloc[c, j % P, dstcol_a[s] + j // P] = e_lane[e0:e1]
                idxA[c, a_base[s] * P + j] = e_src[e0:e1].astype(np.int16)
            else:
                assert m <= KB[s] * P, (bid, m, KB[s] * P)
                dstloc[c, j % P, dstcol_b[s] + j // P] = e_lane[e0:e1]
                idxB[c, b_base[s] * P + j] = (e_src[e0:e1] - VHALF).astype(np.int16)

    return dict(bins=bins, caps=caps, KA=KA, KB=KB, TA=TA, TB=TB,
                T_tot=T_tot, batches=batches, dstloc=dstloc,
                idxA=idxA, idxB=idxB)


def _wrap16(idx_flat):
    n = idx_flat.shape[0]
    arr = np.empty((16, n // 16), dtype=np.int16)
    j = np.arange(n)
    arr[j % 16, j // 16] = idx_flat
    return np.tile(arr, (8, 1))


def _build_nc(plan, repeat=1):
    TA, TB, T_tot = plan["TA"], plan["TB"], plan["T_tot"]
    # batch 0's index columns are stored first, in their own small region,
    # so the first gather calls only wait on a ~0.6us DMA instead of the
    # whole 10KB/partition index table
    ka0 = plan["batches"][0]["ka"]
    kb0 = plan["batches"][0]["kb"]
    plan["ka0"], plan["kb0"] = ka0, kb0
    # const fp32 column layout: [idxA_b0 | idxB_b0][idxA_rest | idxB_rest |
    # W^T | bias]
    c_ia0 = 0                               # idxA batch0 int16 [P, ka0*8]
    c_ib0 = c_ia0 + ka0 * 4                 # idxB batch0 int16 [P, kb0*8]
    c_first = c_ib0 + kb0 * 4               # end of first region
    c_ia = c_first                          # idxA rest int16
    c_ib = c_ia + (TA - ka0) * 4            # idxB rest int16
    c_wt = c_ib + (TB - kb0) * 4            # W^T bf16 [P, P]
    c_brow = c_wt + P // 2                  # bias row bf16 [1, P]
    c_tot = c_brow + P // 2
    plan["c_layout"] = (c_ia0, c_ib0, c_first, c_ia, c_ib, c_wt, c_brow, c_tot)

    nc = bacc.Bacc("TRN2", num_swdge_queues=NQ)
    featA = nc.declare_dram_parameter("featA", [VHALF, P], BF16, isOutput=False)
    featB = nc.declare_dram_parameter("featB", [N_NODES - VHALF, P], BF16,
                                      isOutput=False)
    oneh_d = nc.declare_dram_parameter("oneh", [P, T_tot, P], BF16,
                                       isOutput=False)
    consts = nc.declare_dram_parameter("consts", [P, c_tot], F32, isOutput=False)
    out = nc.declare_dram_parameter("out", [W_SLOTS * P, P], F32, isOutput=True)

    with tile.TileContext(nc) as tc:
        with (
            tc.tile_pool(name="const", bufs=1) as const_pool,
            tc.tile_pool(name="msgs", bufs=3) as msgs_pool,
            tc.tile_pool(name="oneh", bufs=3) as oneh_pool,
            tc.tile_pool(name="outp", bufs=3) as out_pool,
            tc.tile_pool(name="psum", bufs=4, space="PSUM") as psum_pool,
        ):
            # three-way consts load: batch 0's indices land first so the
            # gather stream starts immediately
            cs0 = const_pool.tile([P, c_first], F32, tag="cs_b0")
            nc.sync.dma_start(out=cs0[:], in_=consts[:, 0:c_first])
            cs = const_pool.tile([P, c_wt - c_first], F32, tag="cs_idx")
            nc.sync.dma_start(out=cs[:], in_=consts[:, c_first:c_wt])
            csm = const_pool.tile([P, c_tot - c_wt], F32, tag="cs_misc")
            nc.sync.dma_start(out=csm[:], in_=consts[:, c_wt:c_tot])
            idxA0_sb = cs0[:, c_ia0:c_ib0].bitcast(I16)
            idxB0_sb = cs0[:, c_ib0:c_first].bitcast(I16)
            idxA_sb = cs[:, 0:c_ib - c_ia].bitcast(I16)
            idxB_sb = cs[:, c_ib - c_ia:].bitcast(I16)
            wt_sb = csm[:, 0:c_brow - c_wt].bitcast(BF16)
            brow_sb = csm[0:1, c_brow - c_wt:].bitcast(BF16)
            ones_sb = const_pool.tile([1, P], BF16)
            nc.vector.memset(ones_sb[:], 1.0)

            gq = [0]
            _rep_batches = [bt for _ in range(repeat) for bt in plan["batches"]]

            def gather_calls(msgs, coff, k_tiles, feat, idx_sb, flat0):
                # chunk [0, k_tiles) into CALL_TILES-tile calls (full tiles
                # first; ring/packet capacity caps a call at 1024 indices)
                sizes = [min(CALL_TILES, k_tiles - o)
                         for o in range(0, k_tiles, CALL_TILES)]
                off = 0
                for nk in sizes:
                    nc.gpsimd.dma_gather(
                        out_ap=msgs[:, coff + off:coff + off + nk, :],
                        in_ap=feat[:],
                        idxs_ap=idx_sb[:, (flat0 + off) * 8:(flat0 + off + nk) * 8],
                        num_idxs=nk * P,
                        num_idxs_reg=nk * P,
                        elem_size=P,
                        queue_num=gq[0] % NQ,
                    )
                    gq[0] += 1
                    off += nk

            for bt in _rep_batches:
                ka_b, kb_b = bt["ka"], bt["kb"]
                k_b = ka_b + kb_b
                msgs = msgs_pool.tile([P, k_b, P], BF16, tag="msgs")
                if bt["a0"] == 0:     # batch 0: its own early-loaded region
                    gather_calls(msgs, 0, ka_b, featA, idxA0_sb, 0)
                    gather_calls(msgs, ka_b, kb_b, featB, idxB0_sb, 0)
                else:
                    gather_calls(msgs, 0, ka_b, featA, idxA_sb,
                                 bt["a0"] - plan["ka0"])
                    gather_calls(msgs, ka_b, kb_b, featB, idxB_sb,
                                 bt["b0"] - plan["kb0"])

                onehot = oneh_pool.tile([P, k_b, P], BF16, tag="onehot")
                # host-precomputed one-hot, streamed via HWDGE (keeps DVE off
                # the SBUF port GpSimd's descriptor rings live on). Chunked
                # <=16 tiles per DMA so the prefetch trickles instead of a
                # 2.7MB burst that stalls the gather-ring drains on the
                # shared SDMA engines.
                for c0 in range(0, k_b, 16):
                    c1 = min(c0 + 16, k_b)
                    nc.sync.dma_start(
                        out=onehot[:, c0:c1, :],
                        in_=oneh_d[:, bt["t_base"] + c0:bt["t_base"] + c1, :])
                # one output DMA per batch (windows are contiguous out rows):
                # 7 HWDGE dispatches instead of 49 shortens the kernel tail
                nwin = len(bt["wins"])
                w0 = bt["wins"][0][0]
                out_b = out_pool.tile([P, nwin, P], F32, tag="out_b")
                for wi, (w, cols) in enumerate(bt["wins"]):
                    aggT_ps = psum_pool.tile([P, P], F32, tag="aggT")
                    for i, ccol in enumerate(cols):
                        nc.tensor.matmul(
                            out=aggT_ps[:],
                            lhsT=msgs[:, ccol, :],
                            rhs=onehot[:, ccol, :],
                            start=(i == 0),
                            stop=(i == len(cols) - 1),
                        )
                    aggT_sb = out_pool.tile([P, P], BF16, tag="aggT_sb")
                    nc.scalar.activation(out=aggT_sb[:], in_=aggT_ps[:],
                                         func=mybir.ActivationFunctionType.Copy)
                    out2_ps = psum_pool.tile([P, P], F32, tag="out2")
                    nc.tensor.matmul(out=out2_ps[:], lhsT=aggT_sb[:], rhs=wt_sb,
                                     start=True, stop=False)
                    nc.tensor.matmul(out=out2_ps[:], lhsT=ones_sb[:], rhs=brow_sb,
                                     start=False, stop=True)
                    nc.scalar.activation(out=out_b[:, wi, :], in_=out2_ps[:],
                                         func=mybir.ActivationFunctionType.Relu)
                nc.scalar.dma_start(
                    out=out[w0 * P:(w0 + nwin) * P, :]
                        .rearrange("(w p) f -> p w f", p=P),
                    in_=out_b[:])
    nc.finalize()
    return nc


_CACHE = {}


def _prepare(feature, src, dst, W, b):
    feature = np.asarray(feature, dtype=np.float32)
    W = np.asarray(W, dtype=np.float32)
    b = np.asarray(b, dtype=np.float32)
    key = (hash(np.asarray(src).tobytes()), hash(np.asarray(dst).tobytes()))
    if key not in _CACHE:
        plan = _make_plan(src, dst)
        nc = _build_nc(plan)
        _CACHE.clear()
        _CACHE[key] = (plan, nc)
    plan, nc = _CACHE[key]
    c_ia0, c_ib0, c_first, c_ia, c_ib, c_wt, c_brow, c_tot = plan["c_layout"]
    ka0, kb0 = plan["ka0"], plan["kb0"]
    TA, TB, T_tot = plan["TA"], plan["TB"], plan["T_tot"]
    featA = np.ascontiguousarray(feature[:VHALF].astype(BF))
    featB = np.ascontiguousarray(feature[VHALF:].astype(BF))
    lanes = np.arange(P, dtype=np.float64)
    in_maps = []
    for c in range(NC):
        consts = np.zeros((P, c_tot), dtype=np.float32)

        def put_bf16(col0, arr2d):
            # arr2d [rows, cols] bf16 -> fp32 columns starting at col0
            a = np.asarray(arr2d, dtype=BF)
            rows, cols = a.shape
            pad = (-cols) % 2
            if pad:
                a = np.concatenate([a, np.zeros((rows, pad), BF)], axis=1)
            a = np.ascontiguousarray(a)
            consts[:rows, col0:col0 + a.shape[1] // 2] = a.view(np.float32)

        wA = _wrap16(plan["idxA"][c]).view(np.float32)
        wB = _wrap16(plan["idxB"][c]).view(np.float32)
        consts[:, c_ia0:c_ib0] = wA[:, :ka0 * 4]
        consts[:, c_ib0:c_first] = wB[:, :kb0 * 4]
        consts[:, c_ia:c_ib] = wA[:, ka0 * 4:]
        consts[:, c_ib:c_wt] = wB[:, kb0 * 4:]
        put_bf16(c_wt, W.T.astype(BF))
        put_bf16(c_brow, b[None, :].astype(BF))
        oneh = (plan["dstloc"][c][:, :, None] == lanes[None, None, :]).astype(BF)
        in_maps.append({"featA": featA, "featB": featB, "consts": consts,
                        "oneh": np.ascontiguousarray(oneh)})
    return plan, nc, in_maps


def _assemble(plan, results):
    out_full = np.zeros((N_NODES, P), dtype=np.float32)
    for c in range(NC):
        oc = results[c]["out"]
        for s in range(W_SLOTS):
            nodes = plan["bins"][c * W_SLOTS + s]
            if len(nodes):
                out_full[nodes] = oc[s * P:s * P + len(nodes)]
    return out_full


def kernel(feature, src, dst, W, b):
    plan, nc, in_maps = _prepare(feature, src, dst, W, b)
    res = run_bass_kernel_spmd(nc, in_maps, list(range(NC)))
    return _assemble(plan, res.results)



# revision 5
# speedup vs baseline: 21.8227x; 1.0881x over previous
"""GNN message passing (copy_src + segment_sum + Linear + ReLU) on 8 TRN2 cores.

v2: paired-row gather + on-device one-hot.

Structure: dst nodes are packed (host side) into 392 windows = 8 cores x 49
slots, <=128 nodes per window, with a per-slot uniform tile capacity C_s
shared by all cores (SPMD-identical instruction stream). Per window, edges
are grouped by src row; rows are laid out in a per-window region of a
per-core HBM table (count-matched adjacent row pairs), so one 512B gather
descriptor (elem_size=256 bf16, elem_step=128 -> rows j, j+1) serves TWO
edges. 512B descriptors dodge the sub-512B SDMA read-modify-write penalty,
so this halves both DMA-engine time and Q7 descriptor-generation time vs
per-edge 256B gathers. Each gather tile [128, 256] is two virtual edge
tiles (A = cols 0:128, B = 128:256).

The per-edge one-hot scatter matrices are built ON DEVICE by a single DVE
tensor_tensor is_equal per batch (iota row vs per-slot dst-lane values,
both broadcast via stride-0 APs) instead of streaming ~21MB/core of
precomputed one-hot from HBM. PE matmuls accumulate
aggT[f, lane] += vtile[e, f].T @ onehot[e, lane] in PSUM (fp32). Node
update per window: ACT copies aggT to bf16, matmul with W^T + K=1 bias
matmul, ACT ReLU, one fp32 output DMA per batch on the ACT HWDGE ring.
Host assembles the full output from the window node lists.

Self-contained: shapes hardcoded for feature[50000,128], src/dst[640000],
W[128,128], b[128].
"""
import numpy as np
import ml_dtypes

import concourse.bacc as bacc
import concourse.tile as tile
from concourse import mybir
from concourse.bass_utils import run_bass_kernel_spmd

P = 128
N_NODES = 50000
N_EDGES = 640000
NC = 8
W_SLOTS = 49
NBINS = NC * W_SLOTS                 # 392 windows
CALL_TILES = 8                       # 1024 descriptors per dma_gather call
NQ = 4                               # SWDGE queues
BATCH_SLOTS = 8
C_TILES = 7                          # gather tiles (128 descs) per window
# table parts: slot ranges, so int16 part-relative row indices stay small
PART_SLOTS = [(0, 16), (16, 32), (32, 49)]

F32 = mybir.dt.float32
BF16 = mybir.dt.bfloat16
I16 = mybir.dt.int16
BF = ml_dtypes.bfloat16


def _pack_nodes(deg, ecap):
    """Assign all nodes to NBINS bins: <=128 nodes and <=ecap edges per bin.
    Vectorized greedy, high-degree first."""
    order = np.argsort(-deg)
    e_left = np.full(NBINS, ecap, dtype=np.int64)
    n_left = np.full(NBINS, P, dtype=np.int64)
    assign = np.empty(N_NODES, dtype=np.int64)
    for node in order:
        d = deg[node]
        feas = (n_left > 0) & (e_left >= d)
        if not feas.any():
            return None
        score = e_left + 0.25 * n_left * (ecap / P)
        score[~feas] = -1e18
        bsel = int(np.argmax(score))
        assign[node] = bsel
        e_left[bsel] -= d
        n_left[bsel] -= 1
    return assign


def _make_plan(src, dst):
    src = np.asarray(src, dtype=np.int64)
    dst = np.asarray(dst, dtype=np.int64)
    deg = np.bincount(dst, minlength=N_NODES)

    dcap = C_TILES * P                       # descriptors per window
    for margin in (40, 80, 160):
        assign = _pack_nodes(deg, 2 * dcap - margin)
        if assign is not None:
            break
    else:
        raise RuntimeError("node packing failed")

    bins = [np.where(assign == b)[0] for b in range(NBINS)]
    node_lane = np.empty(N_NODES, dtype=np.int64)
    for nodes in bins:
        node_lane[nodes] = np.arange(len(nodes))

    # edges sorted by (bin, src) once
    ebin = assign[dst]
    order = np.lexsort((src, ebin))
    e_bin = ebin[order]
    e_src = src[order]
    e_lane = node_lane[dst[order]]
    starts = np.concatenate([[0], np.cumsum(np.bincount(e_bin,
                                                        minlength=NBINS))])

    T_slot = C_TILES                          # uniform tiles per slot
    T_tot = W_SLOTS * T_slot
    part_of_slot = np.empty(W_SLOTS, dtype=np.int64)
    for pi, (s0, s1) in enumerate(PART_SLOTS):
        part_of_slot[s0:s1] = pi

    tables = [[[] for _ in range(len(PART_SLOTS))] for _ in range(NC)]
    idx_flat = np.zeros((NC, T_tot * P), dtype=np.int64)
    dstloc = np.full((NC, P, 2 * T_tot), -1.0, dtype=np.float32)

    for c in range(NC):
        for s in range(W_SLOTS):
            bid = c * W_SLOTS + s
            pi = part_of_slot[s]
            tab = tables[c][pi]
            base = len(tab)
            e0, e1 = starts[bid], starts[bid + 1]
            srcs = e_src[e0:e1]
            lanes = e_lane[e0:e1]
            # group by src (srcs sorted within bin)
            rows, row_start = np.unique(srcs, return_index=True)
            row_cnt = np.diff(np.concatenate([row_start, [len(srcs)]]))
            # count-matched pairing: sort rows by count desc, pair (2k,2k+1)
            ro = np.argsort(-row_cnt, kind="stable")
            rows, row_start, row_cnt = rows[ro], row_start[ro], row_cnt[ro]
            nrows = len(rows)
            if nrows % 2:
                rows = np.concatenate([rows, rows[-1:]])   # dup pad row
                row_start = np.concatenate([row_start, [len(srcs)]])
                row_cnt = np.concatenate([row_cnt, [0]])
                nrows += 1
            tab.extend(rows.tolist())
            d = 0
            t_base = s * T_slot
            for k in range(0, nrows, 2):
                ca, cb = int(row_cnt[k]), int(row_cnt[k + 1])
                la = lanes[row_start[k]:row_start[k] + ca]
                lb = lanes[row_start[k + 1]:row_start[k + 1] + cb]
                for i in range(max(ca, cb)):
                    t, p = t_base + d // P, d % P
                    idx_flat[c, t * P + p] = base + k
                    if i < ca:
                        dstloc[c, p, 2 * t] = la[i]
                    if i < cb:
                        dstloc[c, p, 2 * t + 1] = lb[i]
                    d += 1
            assert d <= dcap, (c, s, d, dcap)
            # remaining descriptors in this window's tiles stay idx=0/-1

    R = [max(len(tables[c][pi]) for c in range(NC)) + 2
         for pi in range(len(PART_SLOTS))]

    batches = []
    s = 0
    while s < W_SLOTS:
        s1 = min(s + BATCH_SLOTS, W_SLOTS)
        batches.append(dict(slots=list(range(s, s1)),
                            t_base=s * T_slot,
                            T_b=(s1 - s) * T_slot,
                            part=int(part_of_slot[s])))
        assert part_of_slot[s] == part_of_slot[s1 - 1]
        s = s1

    return dict(bins=bins, tables=tables, R=R, idx_flat=idx_flat,
                dstloc=dstloc, T_tot=T_tot, T_slot=T_slot, batches=batches)


def _wrap16(idx_flat):
    n = idx_flat.shape[0]
    arr = np.empty((16, n // 16), dtype=np.int16)
    j = np.arange(n)
    arr[j % 16, j // 16] = idx_flat
    return np.tile(arr, (8, 1))


def _overlap_ap(t, rows):
    """AP over a [rows+2, P] table reading 256 elems per row step of 128."""
    ap = t[:]
    v = ap.ap
    v[0] = (P, rows)
    v[1] = (1, 2 * P)
    ap.ap = v
    return ap


def _build_nc(plan, repeat=1):
    T_tot, T_slot = plan["T_tot"], plan["T_slot"]
    b0 = plan["batches"][0]
    T_b0 = b0["T_b"]
    # consts fp32 column layout:
    # [idx_b0 | idx_rest | dstloc | iota | W^T | bias]
    c_i0 = 0                                  # idx batch0 int16 [P, T_b0*8]
    c_ir = c_i0 + T_b0 * 4                    # idx rest
    c_dl = c_ir + (T_tot - T_b0) * 4          # dstloc f32 [P, 2*T_tot]
    c_io = c_dl + 2 * T_tot                   # iota f32 [P, P]
    c_wt = c_io + P                           # W^T bf16 [P, P]
    c_br = c_wt + P // 2                      # bias row bf16 [1, P]
    c_tot = c_br + P // 2
    plan["c_layout"] = (c_i0, c_ir, c_dl, c_io, c_wt, c_br, c_tot)

    nc = bacc.Bacc("TRN2", num_swdge_queues=NQ)
    featP = [nc.declare_dram_parameter(f"featP{k}", [plan["R"][k] + 2, P],
                                       BF16, isOutput=False)
             for k in range(len(PART_SLOTS))]
    consts = nc.declare_dram_parameter("consts", [P, c_tot], F32,
                                       isOutput=False)
    out = nc.declare_dram_parameter("out", [W_SLOTS * P, P], F32,
                                    isOutput=True)
    feat_aps = [_overlap_ap(featP[k], plan["R"][k]) for k in
                range(len(PART_SLOTS))]

    with tile.TileContext(nc) as tc:
        with (
            tc.tile_pool(name="const", bufs=1) as const_pool,
            tc.tile_pool(name="msgs", bufs=3) as msgs_pool,
            tc.tile_pool(name="oneh", bufs=3) as oneh_pool,
            tc.tile_pool(name="outp", bufs=3) as out_pool,
            tc.tile_pool(name="psum", bufs=4, space="PSUM") as psum_pool,
        ):
            # batch 0's indices land first so gathers start immediately
            cs0 = const_pool.tile([P, c_ir - c_i0], F32, tag="cs_b0")
            nc.sync.dma_start(out=cs0[:], in_=consts[:, c_i0:c_ir])
            cs = const_pool.tile([P, c_wt - c_ir], F32, tag="cs_rest")
            nc.sync.dma_start(out=cs[:], in_=consts[:, c_ir:c_wt])
            csm = const_pool.tile([P, c_tot - c_wt], F32, tag="cs_misc")
            nc.sync.dma_start(out=csm[:], in_=consts[:, c_wt:c_tot])
            idx0_sb = cs0[:].bitcast(I16)
            idxr_sb = cs[:, 0:c_dl - c_ir].bitcast(I16)
            dstloc_sb = cs[:, c_dl - c_ir:c_io - c_ir]
            iota_sb = cs[:, c_io - c_ir:c_wt - c_ir]
            wt_sb = csm[:, 0:c_br - c_wt].bitcast(BF16)
            brow_sb = csm[0:1, c_br - c_wt:].bitcast(BF16)
            ones_sb = const_pool.tile([1, P], BF16)
            nc.vector.memset(ones_sb[:], 1.0)

            gq = [0]
            _rep_batches = [bt for _ in range(repeat)
                            for bt in plan["batches"]]

            for bt in _rep_batches:
                T_b = bt["T_b"]
                t0 = bt["t_base"]
                fap = feat_aps[bt["part"]]
                msgs = msgs_pool.tile([P, T_b, 2 * P], BF16, tag="msgs")
                for off in range(0, T_b, CALL_TILES):
                    nk = min(CALL_TILES, T_b - off)
                    if t0 == 0:
                        ia = idx0_sb[:, (t0 + off) * 8:(t0 + off + nk) * 8]
                    else:
                        ia = idxr_sb[:, (t0 - T_b0 + off) * 8:
                                     (t0 - T_b0 + off + nk) * 8]
                    nc.gpsimd.dma_gather(
                        out_ap=msgs[:, off:off + nk, :],
                        in_ap=fap,
                        idxs_ap=ia,
                        num_idxs=nk * P,
                        num_idxs_reg=nk * P,
                        elem_size=2 * P,
                        elem_step=P,
                        queue_num=gq[0] % NQ,
                    )
                    gq[0] += 1

                onehot = oneh_pool.tile([P, 2 * T_b, P], BF16, tag="onehot")
                nc.vector.tensor_tensor(
                    out=onehot[:],
                    in0=iota_sb.unsqueeze(1).broadcast_to([P, 2 * T_b, P]),
                    in1=dstloc_sb[:, 2 * t0:2 * (t0 + T_b)]
                        .unsqueeze(2).broadcast_to([P, 2 * T_b, P]),
                    op=mybir.AluOpType.is_equal,
                )

                nwin = len(bt["slots"])
                w0 = bt["slots"][0]
                out_b = out_pool.tile([P, nwin, P], F32, tag="out_b")
                for wi, w in enumerate(bt["slots"]):
                    aggT_ps = psum_pool.tile([P, P], F32, tag="aggT")
                    tloc0 = (w - w0) * T_slot
                    for i in range(T_slot):
                        t = tloc0 + i
                        nc.tensor.matmul(
                            out=aggT_ps[:],
                            lhsT=msgs[:, t, 0:P],
                            rhs=onehot[:, 2 * t, :],
                            start=(i == 0),
                            stop=False,
                        )
                        nc.tensor.matmul(
                            out=aggT_ps[:],
                            lhsT=msgs[:, t, P:2 * P],
                            rhs=onehot[:, 2 * t + 1, :],
                            start=False,
                            stop=(i == T_slot - 1),
                        )
                    aggT_sb = out_pool.tile([P, P], BF16, tag="aggT_sb")
                    nc.scalar.activation(
                        out=aggT_sb[:], in_=aggT_ps[:],
                        func=mybir.ActivationFunctionType.Copy)
                    out2_ps = psum_pool.tile([P, P], F32, tag="out2")
                    nc.tensor.matmul(out=out2_ps[:], lhsT=aggT_sb[:],
                                     rhs=wt_sb, start=True, stop=False)
                    nc.tensor.matmul(out=out2_ps[:], lhsT=ones_sb[:],
                                     rhs=brow_sb, start=False, stop=True)
                    nc.scalar.activation(
                        out=out_b[:, wi, :], in_=out2_ps[:],
                        func=mybir.ActivationFunctionType.Relu)
                nc.scalar.dma_start(
                    out=out[w0 * P:(w0 + nwin) * P, :]
                        .rearrange("(w p) f -> p w f", p=P),
                    in_=out_b[:])
    nc.finalize()
    return nc


_CACHE = {}


def _prepare(feature, src, dst, W, b):
    feature = np.asarray(feature, dtype=np.float32)
    W = np.asarray(W, dtype=np.float32)
    b = np.asarray(b, dtype=np.float32)
    key = (hash(np.asarray(src).tobytes()), hash(np.asarray(dst).tobytes()))
    if key not in _CACHE:
        plan = _make_plan(src, dst)
        nc = _build_nc(plan)
        _CACHE.clear()
        _CACHE[key] = (plan, nc)
    plan, nc = _CACHE[key]
    c_i0, c_ir, c_dl, c_io, c_wt, c_br, c_tot = plan["c_layout"]
    T_tot = plan["T_tot"]
    T_b0 = plan["batches"][0]["T_b"]
    featbf = feature.astype(BF)

    def put_bf16(consts, col0, arr2d):
        a = np.asarray(arr2d, dtype=BF)
        rows, cols = a.shape
        pad = (-cols) % 2
        if pad:
            a = np.concatenate([a, np.zeros((rows, pad), BF)], axis=1)
        a = np.ascontiguousarray(a)
        consts[:rows, col0:col0 + a.shape[1] // 2] = a.view(np.float32)

    in_maps = []
    for c in range(NC):
        consts = np.zeros((P, c_tot), dtype=np.float32)
        wi = _wrap16(plan["idx_flat"][c].astype(np.int16)).view(np.float32)
        consts[:, c_i0:c_ir] = wi[:, :T_b0 * 4]
        consts[:, c_ir:c_dl] = wi[:, T_b0 * 4:]
        consts[:, c_dl:c_io] = plan["dstloc"][c]
        consts[:, c_io:c_wt] = np.tile(np.arange(P, dtype=np.float32), (P, 1))
        put_bf16(consts, c_wt, W.T.astype(BF))
        put_bf16(consts, c_br, b[None, :].astype(BF))
        im = {"consts": consts}
        for k in range(len(PART_SLOTS)):
            tab = np.zeros((plan["R"][k] + 2, P), dtype=BF)
            rows = np.asarray(plan["tables"][c][k], dtype=np.int64)
            if len(rows):
                tab[:len(rows)] = featbf[rows]
            im[f"featP{k}"] = tab
        in_maps.append(im)
    return plan, nc, in_maps


def _assemble(plan, results):
    out_full = np.zeros((N_NODES, P), dtype=np.float32)
    for c in range(NC):
        oc = results[c]["out"]
        for s in range(W_SLOTS):
            nodes = plan["bins"][c * W_SLOTS + s]
            if len(nodes):
                out_full[nodes] = oc[s * P:s * P + len(nodes)]
    return out_full


def kernel(feature, src, dst, W, b):
    plan, nc, in_maps = _prepare(feature, src, dst, W, b)
    res = run_bass_kernel_spmd(nc, in_maps, list(range(NC)))
    return _assemble(plan, res.results)


# revision 8
# speedup vs baseline: 68.6044x; 3.1437x over previous
"""GNN message passing (copy_src + segment_sum + Linear + ReLU) on 8 TRN2 cores.

v2: paired-row gather + on-device one-hot.

Structure: dst nodes are packed (host side) into 392 windows = 8 cores x 49
slots, <=128 nodes per window, with a per-slot uniform tile capacity C_s
shared by all cores (SPMD-identical instruction stream). Per window, edges
are grouped by src row; rows are laid out in a per-window region of a
per-core HBM table (count-matched adjacent row pairs), so one 512B gather
descriptor (elem_size=256 bf16, elem_step=128 -> rows j, j+1) serves TWO
edges. 512B descriptors dodge the sub-512B SDMA read-modify-write penalty,
so this halves both DMA-engine time and Q7 descriptor-generation time vs
per-edge 256B gathers. Each gather tile [128, 256] is two virtual edge
tiles (A = cols 0:128, B = 128:256).

The per-edge one-hot scatter matrices are built ON DEVICE by a single DVE
tensor_tensor is_equal per batch (iota row vs per-slot dst-lane values,
both broadcast via stride-0 APs) instead of streaming ~21MB/core of
precomputed one-hot from HBM. PE matmuls accumulate
aggT[f, lane] += vtile[e, f].T @ onehot[e, lane] in PSUM (fp32). Node
update per window: ACT copies aggT to bf16, matmul with W^T + K=1 bias
matmul, ACT ReLU, one fp32 output DMA per batch on the ACT HWDGE ring.
Host assembles the full output from the window node lists.

Self-contained: shapes hardcoded for feature[50000,128], src/dst[640000],
W[128,128], b[128].
"""
import numpy as np
import ml_dtypes

import concourse.bacc as bacc
import concourse.tile as tile
from concourse import mybir
from concourse.bass_utils import run_bass_kernel_spmd

P = 128
N_NODES = 50000
N_EDGES = 640000
NC = 8
W_SLOTS = 49
NBINS = NC * W_SLOTS                 # 392 windows
CALL_TILES = 8                       # 1024 descriptors per dma_gather call
NQ = 4                               # SWDGE queues
BATCH_SLOTS = 8
C_TILES = 7                          # gather tiles (128 descs) per window
# table parts: slot ranges, so int16 part-relative row indices stay small
PART_SLOTS = [(0, 16), (16, 32), (32, 49)]

F32 = mybir.dt.float32
BF16 = mybir.dt.bfloat16
I16 = mybir.dt.int16
BF = ml_dtypes.bfloat16


def _pack_nodes(deg, ecap):
    """Assign all nodes to NBINS bins: <=128 nodes and <=ecap edges per bin.
    Vectorized greedy, high-degree first."""
    order = np.argsort(-deg)
    e_left = np.full(NBINS, ecap, dtype=np.int64)
    n_left = np.full(NBINS, P, dtype=np.int64)
    assign = np.empty(N_NODES, dtype=np.int64)
    for node in order:
        d = deg[node]
        feas = (n_left > 0) & (e_left >= d)
        if not feas.any():
            return None
        score = e_left + 0.25 * n_left * (ecap / P)
        score[~feas] = -1e18
        bsel = int(np.argmax(score))
        assign[node] = bsel
        e_left[bsel] -= d
        n_left[bsel] -= 1
    return assign


def _make_plan(src, dst):
    src = np.asarray(src, dtype=np.int64)
    dst = np.asarray(dst, dtype=np.int64)
    deg = np.bincount(dst, minlength=N_NODES)

    dcap = C_TILES * P                       # descriptors per window
    for margin in (40, 80, 160):
        assign = _pack_nodes(deg, 2 * dcap - margin)
        if assign is not None:
            break
    else:
        raise RuntimeError("node packing failed")

    bins = [np.where(assign == b)[0] for b in range(NBINS)]
    node_lane = np.empty(N_NODES, dtype=np.int64)
    for nodes in bins:
        node_lane[nodes] = np.arange(len(nodes))

    # edges sorted by (bin, src) once
    ebin = assign[dst]
    order = np.lexsort((src, ebin))
    e_bin = ebin[order]
    e_src = src[order]
    e_lane = node_lane[dst[order]]
    starts = np.concatenate([[0], np.cumsum(np.bincount(e_bin,
                                                        minlength=NBINS))])

    T_slot = C_TILES                          # uniform tiles per slot
    T_tot = W_SLOTS * T_slot
    part_of_slot = np.empty(W_SLOTS, dtype=np.int64)
    for pi, (s0, s1) in enumerate(PART_SLOTS):
        part_of_slot[s0:s1] = pi

    tables = [[[] for _ in range(len(PART_SLOTS))] for _ in range(NC)]
    idx_flat = np.zeros((NC, T_tot * P), dtype=np.int64)
    dstloc = np.full((NC, P, 2 * T_tot), -1.0, dtype=np.float32)

    for c in range(NC):
        for s in range(W_SLOTS):
            bid = c * W_SLOTS + s
            pi = part_of_slot[s]
            tab = tables[c][pi]
            base = len(tab)
            e0, e1 = starts[bid], starts[bid + 1]
            srcs = e_src[e0:e1]
            lanes = e_lane[e0:e1]
            # group by src (srcs sorted within bin)
            rows, row_start = np.unique(srcs, return_index=True)
            row_cnt = np.diff(np.concatenate([row_start, [len(srcs)]]))
            # count-matched pairing: sort rows by count desc, pair (2k,2k+1)
            ro = np.argsort(-row_cnt, kind="stable")
            rows, row_start, row_cnt = rows[ro], row_start[ro], row_cnt[ro]
            nrows = len(rows)
            if nrows % 2:
                rows = np.concatenate([rows, rows[-1:]])   # dup pad row
                row_start = np.concatenate([row_start, [len(srcs)]])
                row_cnt = np.concatenate([row_cnt, [0]])
                nrows += 1
            tab.extend(rows.tolist())
            d = 0
            t_base = s * T_slot
            for k in range(0, nrows, 2):
                ca, cb = int(row_cnt[k]), int(row_cnt[k + 1])
                la = lanes[row_start[k]:row_start[k] + ca]
                lb = lanes[row_start[k + 1]:row_start[k + 1] + cb]
                for i in range(max(ca, cb)):
                    t, p = t_base + d // P, d % P
                    idx_flat[c, t * P + p] = base + k
                    if i < ca:
                        dstloc[c, p, 2 * t] = la[i]
                    if i < cb:
                        dstloc[c, p, 2 * t + 1] = lb[i]
                    d += 1
            assert d <= dcap, (c, s, d, dcap)
            # remaining descriptors in this window's tiles stay idx=0/-1

    R = [max(len(tables[c][pi]) for c in range(NC)) + 2
         for pi in range(len(PART_SLOTS))]

    batches = []
    s = 0
    while s < W_SLOTS:
        s1 = min(s + BATCH_SLOTS, W_SLOTS)
        batches.append(dict(slots=list(range(s, s1)),
                            t_base=s * T_slot,
                            T_b=(s1 - s) * T_slot,
                            part=int(part_of_slot[s])))
        assert part_of_slot[s] == part_of_slot[s1 - 1]
        s = s1

    return dict(bins=bins, tables=tables, R=R, idx_flat=idx_flat,
                dstloc=dstloc, T_tot=T_tot, T_slot=T_slot, batches=batches)


def _wrap16(idx_flat):
    n = idx_flat.shape[0]
    arr = np.empty((16, n // 16), dtype=np.int16)
    j = np.arange(n)
    arr[j % 16, j // 16] = idx_flat
    return np.tile(arr, (8, 1))


def _overlap_ap(t, rows):
    """AP over a [rows+2, P] table reading 256 elems per row step of 128."""
    ap = t[:]
    v = ap.ap
    v[0] = (P, rows)
    v[1] = (1, 2 * P)
    ap.ap = v
    return ap


def _build_nc(plan, repeat=1):
    T_tot, T_slot = plan["T_tot"], plan["T_slot"]
    b0 = plan["batches"][0]
    T_b0 = b0["T_b"]
    # consts fp32 column layout:
    # [idx_b0 | idx_rest | dstloc | iota | W^T | bias]
    c_i0 = 0                                  # idx batch0 int16 [P, T_b0*8]
    c_ir = c_i0 + T_b0 * 4                    # idx rest
    c_dl = c_ir + (T_tot - T_b0) * 4          # dstloc f32 [P, 2*T_tot]
    c_io = c_dl + 2 * T_tot                   # iota f32 [P, P]
    c_wt = c_io + P                           # W^T bf16 [P, P]
    c_br = c_wt + P // 2                      # bias row bf16 [1, P]
    c_tot = c_br + P // 2
    plan["c_layout"] = (c_i0, c_ir, c_dl, c_io, c_wt, c_br, c_tot)

    nc = bacc.Bacc("TRN2", num_swdge_queues=NQ)
    featP = [nc.declare_dram_parameter(f"featP{k}", [plan["R"][k] + 2, P],
                                       BF16, isOutput=False)
             for k in range(len(PART_SLOTS))]
    consts = nc.declare_dram_parameter("consts", [P, c_tot], F32,
                                       isOutput=False)
    out = nc.declare_dram_parameter("out", [W_SLOTS * P, P], F32,
                                    isOutput=True)
    feat_aps = [_overlap_ap(featP[k], plan["R"][k]) for k in
                range(len(PART_SLOTS))]

    with tile.TileContext(nc) as tc:
        with (
            tc.tile_pool(name="const", bufs=1) as const_pool,
            tc.tile_pool(name="msgs", bufs=3) as msgs_pool,
            tc.tile_pool(name="oneh", bufs=3) as oneh_pool,
            tc.tile_pool(name="outp", bufs=3) as out_pool,
            tc.tile_pool(name="psum", bufs=4, space="PSUM") as psum_pool,
        ):
            # batch 0's indices land first so gathers start immediately
            cs0 = const_pool.tile([P, c_ir - c_i0], F32, tag="cs_b0")
            nc.sync.dma_start(out=cs0[:], in_=consts[:, c_i0:c_ir])
            cs = const_pool.tile([P, c_wt - c_ir], F32, tag="cs_rest")
            nc.sync.dma_start(out=cs[:], in_=consts[:, c_ir:c_wt])
            csm = const_pool.tile([P, c_tot - c_wt], F32, tag="cs_misc")
            nc.sync.dma_start(out=csm[:], in_=consts[:, c_wt:c_tot])
            idx0_sb = cs0[:].bitcast(I16)
            idxr_sb = cs[:, 0:c_dl - c_ir].bitcast(I16)
            dstloc_sb = cs[:, c_dl - c_ir:c_io - c_ir]
            iota_sb = cs[:, c_io - c_ir:c_wt - c_ir]
            wt_sb = csm[:, 0:c_br - c_wt].bitcast(BF16)
            brow_sb = csm[0:1, c_br - c_wt:].bitcast(BF16)
            ones_sb = const_pool.tile([1, P], BF16)
            nc.vector.memset(ones_sb[:], 1.0)
            import os as _os
            _no_dve = bool(_os.environ.get("KOPT_NO_DVE"))
            if _no_dve:
                oh_const = const_pool.tile([P, 2 * 8 * T_slot, P], BF16,
                                           tag="oh_const")
                nc.vector.memset(oh_const[:], 0.0)
            _no_gather = bool(_os.environ.get("KOPT_NO_GATHER"))
            if _no_gather:
                msgs_const = const_pool.tile([P, 8 * T_slot, 2 * P], BF16,
                                             tag="msgs_const")
                nc.vector.memset(msgs_const[:], 0.0)

            gq = [0]
            _rep_batches = [bt for _ in range(repeat)
                            for bt in plan["batches"]]

            for bt in _rep_batches:
                T_b = bt["T_b"]
                t0 = bt["t_base"]
                fap = feat_aps[bt["part"]]
                msgs = (msgs_const if _no_gather else
                        msgs_pool.tile([P, T_b, 2 * P], BF16, tag="msgs"))
                for off in ([] if _no_gather else range(0, T_b, CALL_TILES)):
                    nk = min(CALL_TILES, T_b - off)
                    if t0 == 0:
                        ia = idx0_sb[:, (t0 + off) * 8:(t0 + off + nk) * 8]
                    else:
                        ia = idxr_sb[:, (t0 - T_b0 + off) * 8:
                                     (t0 - T_b0 + off + nk) * 8]
                    nc.gpsimd.dma_gather(
                        out_ap=msgs[:, off:off + nk, :],
                        in_ap=fap,
                        idxs_ap=ia,
                        num_idxs=nk * P,
                        num_idxs_reg=nk * P,
                        elem_size=2 * P,
                        elem_step=P,
                        queue_num=gq[0] % NQ,
                    )
                    gq[0] += 1

                if _no_dve:
                    onehot = oh_const
                else:
                    onehot = oneh_pool.tile([P, 2 * T_b, P], BF16,
                                            tag="onehot")
                    nc.vector.tensor_tensor(
                        out=onehot[:],
                        in0=iota_sb.unsqueeze(1).broadcast_to([P, 2 * T_b, P]),
                        in1=dstloc_sb[:, 2 * t0:2 * (t0 + T_b)]
                            .unsqueeze(2).broadcast_to([P, 2 * T_b, P]),
                        op=mybir.AluOpType.is_equal,
                    )

                nwin = len(bt["slots"])
                w0 = bt["slots"][0]
                out_b = out_pool.tile([P, nwin, P], F32, tag="out_b")
                for wi, w in enumerate(bt["slots"]):
                    aggT_ps = psum_pool.tile([P, P], F32, tag="aggT")
                    tloc0 = (w - w0) * T_slot
                    for i in range(T_slot):
                        t = tloc0 + i
                        nc.tensor.matmul(
                            out=aggT_ps[:],
                            lhsT=msgs[:, t, 0:P],
                            rhs=onehot[:, 2 * t, :],
                            start=(i == 0),
                            stop=False,
                        )
                        nc.tensor.matmul(
                            out=aggT_ps[:],
                            lhsT=msgs[:, t, P:2 * P],
                            rhs=onehot[:, 2 * t + 1, :],
                            start=False,
                            stop=(i == T_slot - 1),
                        )
                    aggT_sb = out_pool.tile([P, P], BF16, tag="aggT_sb")
                    nc.scalar.activation(
                        out=aggT_sb[:], in_=aggT_ps[:],
                        func=mybir.ActivationFunctionType.Copy)
                    out2_ps = psum_pool.tile([P, P], F32, tag="out2")
                    nc.tensor.matmul(out=out2_ps[:], lhsT=aggT_sb[:],
                                     rhs=wt_sb, start=True, stop=False)
                    nc.tensor.matmul(out=out2_ps[:], lhsT=ones_sb[:],
                                     rhs=brow_sb, start=False, stop=True)
                    nc.scalar.activation(
                        out=out_b[:, wi, :], in_=out2_ps[:],
                        func=mybir.ActivationFunctionType.Relu)
                nc.scalar.dma_start(
                    out=out[w0 * P:(w0 + nwin) * P, :]
                        .rearrange("(w p) f -> p w f", p=P),
                    in_=out_b[:])
    nc.finalize()
    return nc


_CACHE = {}


def _prepare(feature, src, dst, W, b):
    feature = np.asarray(feature, dtype=np.float32)
    W = np.asarray(W, dtype=np.float32)
    b = np.asarray(b, dtype=np.float32)
    key = (hash(np.asarray(src).tobytes()), hash(np.asarray(dst).tobytes()))
    if key not in _CACHE:
        plan = _make_plan(src, dst)
        nc = _build_nc(plan)
        _CACHE.clear()
        _CACHE[key] = (plan, nc)
    plan, nc = _CACHE[key]
    c_i0, c_ir, c_dl, c_io, c_wt, c_br, c_tot = plan["c_layout"]
    T_tot = plan["T_tot"]
    T_b0 = plan["batches"][0]["T_b"]
    featbf = feature.astype(BF)

    def put_bf16(consts, col0, arr2d):
        a = np.asarray(arr2d, dtype=BF)
        rows, cols = a.shape
        pad = (-cols) % 2
        if pad:
            a = np.concatenate([a, np.zeros((rows, pad), BF)], axis=1)
        a = np.ascontiguousarray(a)
        consts[:rows, col0:col0 + a.shape[1] // 2] = a.view(np.float32)

    in_maps = []
    for c in range(NC):
        consts = np.zeros((P, c_tot), dtype=np.float32)
        wi = _wrap16(plan["idx_flat"][c].astype(np.int16)).view(np.float32)
        consts[:, c_i0:c_ir] = wi[:, :T_b0 * 4]
        consts[:, c_ir:c_dl] = wi[:, T_b0 * 4:]
        consts[:, c_dl:c_io] = plan["dstloc"][c]
        consts[:, c_io:c_wt] = np.tile(np.arange(P, dtype=np.float32), (P, 1))
        put_bf16(consts, c_wt, W.T.astype(BF))
        put_bf16(consts, c_br, b[None, :].astype(BF))
        im = {"consts": consts}
        for k in range(len(PART_SLOTS)):
            tab = np.zeros((plan["R"][k] + 2, P), dtype=BF)
            rows = np.asarray(plan["tables"][c][k], dtype=np.int64)
            if len(rows):
                tab[:len(rows)] = featbf[rows]
            im[f"featP{k}"] = tab
        in_maps.append(im)
    return plan, nc, in_maps


def _assemble(plan, results):
    out_full = np.zeros((N_NODES, P), dtype=np.float32)
    for c in range(NC):
        oc = results[c]["out"]
        for s in range(W_SLOTS):
            nodes = plan["bins"][c * W_SLOTS + s]
            if len(nodes):
                out_full[nodes] = oc[s * P:s * P + len(nodes)]
    return out_full


def kernel(feature, src, dst, W, b):
    plan, nc, in_maps = _prepare(feature, src, dst, W, b)
    res = run_bass_kernel_spmd(nc, in_maps, list(range(NC)))
    return _assemble(plan, res.results)
